# revision 13
# baseline (speedup 1.0000x reference)
"""Mixture-of-Experts (top-2 of 8) Trainium2 kernel, expert-parallel over 8 NeuronCores.

Strategy (per the expert-parallel sharding hint):
  Host routing ("all-to-all dispatch"): compute fp32 gating in numpy purely to
    DECIDE which tokens go to which expert/core; gather+transpose the routed
    tokens for each expert, pad to a common capacity C.
  Single device launch (expert-parallel): core e holds expert e's weights and
    its routed tokens. On device it computes
      - gating logits, top-2 selection and renormalized combine weights for
        its tokens (fp32 on the PE + vector/scalar engines) — these are the
        values that scale the output, so gating math lives on device; the
        host's numpy gating only chose the routing (device and host agree
        because both select top-2 from fp32 logits),
      - h^T = gelu(W1^T x^T + b1) and y^T = (W2^T h^T + b2) * w in bf16 with
        fp32 accumulation; biases applied exactly in fp32.
  Host unshard: scatter-add the 8 weighted partial outputs into [T, D].
"""

import os
import sys
import types

import numpy as np
import ml_dtypes

import concourse.bass as bass
import concourse.mybir as mybir
import concourse.tile as tile
from concourse import bacc
from concourse.bass_utils import run_bass_kernel_spmd
from concourse.masks import make_identity

N_CORES = 8
P = 128
B, S, D, H, E = 2, 2048, 1024, 4096, 8
T = B * S
BF16 = ml_dtypes.bfloat16

AF = mybir.ActivationFunctionType
ALU = mybir.AluOpType
AX = mybir.AxisListType
F32 = mybir.dt.float32
BF = mybir.dt.bfloat16


def _install_profile_hook():
    """Register the antenv.axon_hooks NTFF hook this image lacks, so
    BASS_TRACE=1 profiling works. Harmless no-op on failure."""
    try:
        if "antenv.axon_hooks" in sys.modules:
            return
        import antenv
        from trn_agent_boot.trn_boot import _ntff_profile_via_ctypes

        mod = types.ModuleType("antenv.axon_hooks")
        _h = [None]
        mod.set_axon_ntff_profile_hook = lambda h: _h.__setitem__(0, h)
        mod.get_axon_ntff_profile_hook = lambda: _h[0]
        sys.modules["antenv.axon_hooks"] = mod
        antenv.axon_hooks = mod
        so = "/opt/axon/libaxon_pjrt.so"
        if os.path.exists(so):
            mod.set_axon_ntff_profile_hook(_ntff_profile_via_ctypes(so))
    except Exception:
        pass


_install_profile_hook()

_NC_CACHE = {}


def _build_moe_nc(C):
    """Single-launch per-core program: gating + expert FFN over C (padded)
    routed tokens.

    Inputs : xt   [D, C] bf16 — routed tokens, transposed (for the FFN)
             xtf  [D, C] f32  — same tokens in fp32 (for exact gating)
             wg   [D, E] f32  — gating weights (replicated)
             esel [P, E] f32  — one-hot row for this core's expert, replicated
             w1 [D, H] bf16, w2 [H, D] bf16 — this expert's weights
             b1r [P, H/P] f32, b2r [P, D/P] f32 — biases, partition-major
    Output : yt [D, C] f32 — w * (gelu(x W1 + b1) W2 + b2), transposed
    """
    key = ("moe", C)
    if key in _NC_CACHE:
        return _NC_CACHE[key]
    assert C % P == 0
    KD = D // P  # 8 k-tiles over D
    KH = H // P  # 32 k-tiles over H
    CT = C // P  # token tiles
    # W1 dma chunk sizes over H: small first chunks so the PE starts early
    h_chunks = [128, 384] + [512] * 7
    assert sum(h_chunks) == H
    DC = 256  # d columns per W2 dma chunk
    n_off = list(range(0, C, 512))
    n_szs = [min(512, C - o) for o in n_off]
    NCH = len(n_off)

    nc = bacc.Bacc("TRN2", target_bir_lowering=False, debug=False, num_devices=N_CORES)
    xt = nc.dram_tensor("xt", [D, C], BF, kind="ExternalInput")
    xtf = nc.dram_tensor("xtf", [D, C], F32, kind="ExternalInput")
    wg = nc.dram_tensor("wg", [D, E], F32, kind="ExternalInput")
    esel = nc.dram_tensor("esel", [P, E], F32, kind="ExternalInput")
    w1 = nc.dram_tensor("w1", [D, H], BF, kind="ExternalInput")
    w2 = nc.dram_tensor("w2", [H, D], BF, kind="ExternalInput")
    b1r = nc.dram_tensor("b1r", [P, H // P], F32, kind="ExternalInput")
    b2r = nc.dram_tensor("b2r", [P, D // P], F32, kind="ExternalInput")
    yt = nc.dram_tensor("yt", [D, C], F32, kind="ExternalOutput")

    with tile.TileContext(nc) as tc:
        with (
            tc.tile_pool(name="cst", bufs=1) as cst,
            tc.tile_pool(name="w1p", bufs=3) as w1p,
            tc.tile_pool(name="w2p", bufs=2) as w2p,
            tc.tile_pool(name="xfp", bufs=2) as xfp,
            tc.tile_pool(name="wk", bufs=4) as wk,
            tc.tile_pool(name="outp", bufs=3) as outp,
            tc.tile_pool(name="ps", bufs=3, space="PSUM") as ps,
        ):
            # ---- input DMAs, interleaved so the PE can start early ----
            # sync (HWDGE) queue: W1 chunks + xt k-slices, in consumption order
            w1_c0 = w1p.tile([P, KD, h_chunks[0]], BF, tag="w1c", name="w1_c0")
            nc.sync.dma_start(
                w1_c0[:],
                w1.ap()[:, 0 : h_chunks[0]].rearrange("(kd p) h -> p kd h", p=P),
            )
            xt_sb = cst.tile([P, KD, C], BF)
            xt_ap = xt.ap().rearrange("(kd p) c -> p kd c", p=P)
            for kd in range(2):
                nc.sync.dma_start(xt_sb[:, kd, :], xt_ap[:, kd, :])
            w1_c1 = w1p.tile([P, KD, 512], BF, tag="w1c", name="w1_c1")
            nc.sync.dma_start(
                w1_c1[:, :, : h_chunks[1]],
                w1.ap()[:, h_chunks[0] : h_chunks[0] + h_chunks[1]].rearrange(
                    "(kd p) h -> p kd h", p=P
                ),
            )
            for kd in range(2, KD):
                nc.sync.dma_start(xt_sb[:, kd, :], xt_ap[:, kd, :])
            # gpsimd (SWDGE) queue: small or latency-tolerant loads
            b1_sb = cst.tile([P, H // P], F32)
            nc.gpsimd.dma_start(b1_sb[:], b1r.ap())
            b2_sb = cst.tile([P, D // P], F32)
            nc.gpsimd.dma_start(b2_sb[:], b2r.ap())
            esel_sb = cst.tile([P, E], F32)
            nc.gpsimd.dma_start(esel_sb[:], esel.ap())
            wg_sb = cst.tile([P, KD, E], F32)
            nc.gpsimd.dma_start(wg_sb[:], wg.ap().rearrange("(kd p) e -> p kd e", p=P))
            ht_sb = cst.tile([P, KH, C], BF)

            # ---- mm1: ht[h, c] = gelu(sum_d w1[d, h] * xt[d, c] + b1[h]) ----
            h_off = 0
            h_tile = 0
            gelu_insts = []
            last_mm1 = None
            for hc, hsz in enumerate(h_chunks):
                if hc == 0:
                    w1_c = w1_c0
                elif hc == 1:
                    w1_c = w1_c1
                else:
                    w1_c = w1p.tile([P, KD, 512], BF, tag="w1c", name=f"w1_c{hc}")
                    nc.sync.dma_start(
                        w1_c[:, :, :hsz],
                        w1.ap()[:, h_off : h_off + hsz].rearrange(
                            "(kd p) h -> p kd h", p=P
                        ),
                    )
                for hs in range(hsz // P):
                    psum_ts = [
                        ps.tile([P, 512], F32, tag="psmm", name=f"ps1_{h_tile}_{n}")
                        for n in range(NCH)
                    ]
                    for kd in range(KD):
                        for n in range(NCH):
                            last_mm1 = nc.tensor.matmul(
                                psum_ts[n][:, : n_szs[n]],
                                w1_c[:, kd, hs * P : (hs + 1) * P],
                                xt_sb[:, kd, n_off[n] : n_off[n] + n_szs[n]],
                                start=(kd == 0),
                                stop=(kd == KD - 1),
                            )
                    for n in range(NCH):
                        g = nc.scalar.activation(
                            ht_sb[:, h_tile, n_off[n] : n_off[n] + n_szs[n]],
                            psum_ts[n][:, : n_szs[n]],
                            AF.Gelu,
                            bias=b1_sb[:, h_tile : h_tile + 1],
                        )
                        if n == 0:
                            gelu_insts.append(g)
                    h_tile += 1
                h_off += hsz

            # ---- gating: w[c] = renormalized top-2 weight of this core's
            #      expert for token c, from fp32 logits computed on the PE ----
            ident8 = cst.tile([E, E], F32)
            make_identity(nc, ident8[:])
            identp = cst.tile([P, P], F32)
            make_identity(nc, identp[:])
            ones1 = cst.tile([1, P], F32)
            nc.vector.memset(ones1[:], 1.0)

            # logits^T accumulated over k-slices of xtf (streamed, fp32)
            psg = [
                ps.tile([E, 512], F32, tag="psg", bufs=NCH, name=f"psg_{n}")
                for n in range(NCH)
            ]
            xtf_ap = xtf.ap().rearrange("(kd p) c -> p kd c", p=P)
            for kd in range(KD):
                xtf_k = xfp.tile([P, C], F32, tag="xtfk", name=f"xtf_{kd}")
                dma = nc.gpsimd.dma_start(xtf_k[:], xtf_ap[:, kd, :])
                # keep these off the head's HBM bandwidth: start them only
                # once mm1 is well underway
                bass._add_dep_helper(
                    dma.ins,
                    gelu_insts[min(6 + 2 * kd, KH - 2)].ins,
                    sync=True,
                    reason="delay xtf load into mm1 steady state",
                )
                for n in range(NCH):
                    mm = nc.tensor.matmul(
                        psg[n][:, : n_szs[n]],
                        wg_sb[:, kd, :],
                        xtf_k[:, n_off[n] : n_off[n] + n_szs[n]],
                        start=(kd == 0),
                        stop=(kd == KD - 1),
                    )
                    if kd == 0:
                        # keep the fp32 gating matmuls out of the bf16 mm1
                        # stream: contiguous fp32 block runs ~3x faster
                        bass._add_dep_helper(
                            mm.ins,
                            last_mm1.ins,
                            sync=False,
                            reason="gating matmuls after mm1",
                        )
            lt_sb = cst.tile([E, C], F32)
            for n in range(NCH):
                nc.scalar.copy(lt_sb[:, n_off[n] : n_off[n] + n_szs[n]], psg[n][:, : n_szs[n]])
            wrow_sb = cst.tile([1, C], F32)
            for ct in range(CT):
                csl = slice(ct * P, (ct + 1) * P)
                # tokens onto partitions
                pg = ps.tile([P, E], F32, tag="pssm", bufs=2, name=f"pg_{ct}")
                nc.tensor.transpose(pg[:], lt_sb[:, csl], ident8[:])
                logits = wk.tile([P, E], F32, tag="logits")
                nc.scalar.copy(logits[:], pg[:])
                top8 = wk.tile([P, 8], F32, tag="top8")
                nc.vector.max(out=top8[:], in_=logits[:])
                negm1 = wk.tile([P, 1], F32, tag="negm1")
                nc.vector.tensor_scalar_mul(negm1[:], top8[:, 0:1], -1.0)
                mask = wk.tile([P, E], F32, tag="mask")
                nc.vector.tensor_scalar(
                    out=mask[:],
                    in0=logits[:],
                    scalar1=top8[:, 1:2],
                    scalar2=None,
                    op0=ALU.is_ge,
                )
                ex = wk.tile([P, E], F32, tag="ex")
                nc.scalar.activation(ex[:], logits[:], AF.Exp, bias=negm1[:])
                wv = wk.tile([P, E], F32, tag="wv")
                nc.vector.tensor_mul(wv[:], ex[:], mask[:])
                ssum = wk.tile([P, 1], F32, tag="ssum")
                nc.vector.reduce_sum(ssum[:], wv[:], axis=AX.X)
                rec = wk.tile([P, 1], F32, tag="rec")
                nc.vector.reciprocal(rec[:], ssum[:])
                wn = wk.tile([P, E], F32, tag="wn")
                nc.vector.tensor_scalar_mul(wn[:], wv[:], rec[:])
                # select this core's expert column: dot with one-hot
                wsel = wk.tile([P, E], F32, tag="wsel")
                nc.vector.tensor_mul(wsel[:], wn[:], esel_sb[:])
                wtok = wk.tile([P, 1], F32, tag="wtok")
                nc.vector.reduce_sum(wtok[:], wsel[:], axis=AX.X)
                # back to a row vector [1, P]
                pw = ps.tile([1, P], F32, tag="pssm", bufs=2, name=f"pw_{ct}")
                nc.tensor.transpose(pw[:], wtok[:], identp[:])
                nc.scalar.copy(wrow_sb[:, csl], pw[:])
            # broadcast w to all partitions: outer product ones[P,1] @ wrow[1,C]
            wc_sb = cst.tile([P, C], F32)
            for n in range(NCH):
                pwb = ps.tile([P, 512], F32, tag="pssm", bufs=2, name=f"pwb_{n}")
                nc.tensor.matmul(
                    pwb[:, : n_szs[n]],
                    ones1[:],
                    wrow_sb[:, n_off[n] : n_off[n] + n_szs[n]],
                    start=True,
                    stop=True,
                )
                nc.scalar.copy(wc_sb[:, n_off[n] : n_off[n] + n_szs[n]], pwb[:, : n_szs[n]])

            # ---- mm2: yt[d, c] = (sum_h w2[h, d] * ht[h, c] + b2[d]) * wc[c] ----
            for dc in range(D // DC):
                w2_c = w2p.tile([P, KH, DC], BF, tag="w2c", name=f"w2_c{dc}")
                nc.sync.dma_start(
                    w2_c[:],
                    w2.ap()[:, dc * DC : (dc + 1) * DC].rearrange(
                        "(kh p) d -> p kh d", p=P
                    ),
                )
                for dsx in range(DC // P):
                    d_tile = dc * (DC // P) + dsx
                    psum_ts = [
                        ps.tile([P, 512], F32, tag="psmm", name=f"ps2_{d_tile}_{n}")
                        for n in range(NCH)
                    ]
                    for kh in range(KH):
                        for n in range(NCH):
                            nc.tensor.matmul(
                                psum_ts[n][:, : n_szs[n]],
                                w2_c[:, kh, dsx * P : (dsx + 1) * P],
                                ht_sb[:, kh, n_off[n] : n_off[n] + n_szs[n]],
                                start=(kh == 0),
                                stop=(kh == KH - 1),
                            )
                    for n in range(NCH):
                        nsz = n_szs[n]
                        tmp = outp.tile([P, 512], F32, tag="tmp")
                        nc.scalar.activation(
                            tmp[:, :nsz],
                            psum_ts[n][:, :nsz],
                            AF.Identity,
                            bias=b2_sb[:, d_tile : d_tile + 1],
                        )
                        out_t = outp.tile([P, 512], F32, tag="out")
                        nc.vector.tensor_mul(
                            out_t[:, :nsz],
                            tmp[:, :nsz],
                            wc_sb[:, n_off[n] : n_off[n] + nsz],
                        )
                        nc.sync.dma_start(
                            yt.ap().rearrange("(dt p) c -> p dt c", p=P)[
                                :, d_tile, n_off[n] : n_off[n] + nsz
                            ],
                            out_t[:, :nsz],
                        )
    nc.compile()
    _NC_CACHE[key] = nc
    return nc


# results of the most recent kernel() call, for test harness introspection
last_results = {}


def kernel(**inputs):
    x = np.asarray(inputs["x"], np.float32)
    Wg = np.asarray(inputs["Wg"], np.float32)
    W1 = np.asarray(inputs["W1"], np.float32)
    b1 = np.asarray(inputs["b1"], np.float32)
    W2 = np.asarray(inputs["W2"], np.float32)
    b2 = np.asarray(inputs["b2"], np.float32)
    assert x.shape == (B, S, D) and Wg.shape == (D, E)
    assert W1.shape == (E, D, H) and W2.shape == (E, H, D)

    xf = np.ascontiguousarray(x.reshape(T, D))
    core_ids = list(range(N_CORES))

    # ---- Host routing (sharding decision only): fp32 top-2 of fp32 logits.
    # The device recomputes the same fp32 gating for the combine weights; a
    # disagreement would need two logits within ~1 ulp of each other.
    logits = xf @ Wg
    part = np.argpartition(-logits, 2, axis=1)[:, :2]
    idx_list = []
    max_cnt = 1
    for e in range(E):
        idx = np.nonzero((part == e).any(axis=1))[0]
        idx_list.append(idx)
        max_cnt = max(max_cnt, len(idx))
    C = ((max_cnt + P - 1) // P) * P

    # ---- Single expert-parallel launch ----
    ncK = _build_moe_nc(C)
    eye = np.eye(E, dtype=np.float32)
    in_maps = []
    for e in range(E):
        idx = idx_list[e]
        cnt = len(idx)
        xg = xf[idx]
        xtf = np.zeros((D, C), np.float32)
        xtf[:, :cnt] = xg.T
        xt = np.zeros((D, C), BF16)
        xt[:, :cnt] = xg.T.astype(BF16)
        in_maps.append(
            {
                "xt": xt,
                "xtf": xtf,
                "wg": Wg,
                "esel": np.ascontiguousarray(np.broadcast_to(eye[e], (P, E))),
                "w1": np.ascontiguousarray(W1[e].astype(BF16)),
                "w2": np.ascontiguousarray(W2[e].astype(BF16)),
                "b1r": np.ascontiguousarray(b1[e].reshape(H // P, P).T),
                "b2r": np.ascontiguousarray(b2[e].reshape(D // P, P).T),
            }
        )
    res = run_bass_kernel_spmd(ncK, in_maps, core_ids=core_ids)

    # ---- Host unshard: scatter-add weighted partial outputs ----
    out = np.zeros((T, D), np.float32)
    for e in range(E):
        idx = idx_list[e]
        cnt = len(idx)
        if cnt:
            out[idx] += res.results[e]["yt"][:, :cnt].T

    last_results.clear()
    last_results["moe"] = res
    return out.reshape(B, S, D)
